# revision 4
# baseline (speedup 1.0000x reference)
"""Trainium2 Bass kernel for nn_Net_71554155151864 (e3nn-style GNN message-passing).

Transfer-optimized revision: the end-to-end wall clock is dominated by the
~50 MB/s host<->device tunnel, so every byte shipped is minimized:
 - edge features int8 (x32), edge_length/one_hot uint8, sh bf16
 - node table + all weights packed into two blobs, sharded across the 8
   cores and AllGather'ed on device (no 8x replication over the tunnel)
 - per-core graph one-hots shrunk to the 2 local graphs; all small aux
   tensors packed into one uint8 byte-blob input (one transfer)
 - gather indices shipped 16-wide and replicated to 128 partitions by DMA
 - kernel returns only the int8 z-delta (x24, rounds+saturates on DVE);
   the fp32 skip-add with edge_fea happens on host
 - host packing streams each tensor to the device as soon as it is built;
   output shards are fetched per-core with the skip-add overlapped
Compute layout (feature-major, graph-per-core sharding) is unchanged from
the baseline: 2 graphs/core so e3LayerNorm stats stay core-local.
"""
import math
import numpy as np
import ml_dtypes

import jax
import jax.numpy as jnp
from jax.sharding import Mesh, PartitionSpec, NamedSharding
from jax.experimental.shard_map import shard_map

import concourse.bacc as bacc
import concourse.bass as bass
import concourse.mybir as mybir
import concourse.tile as tile
from concourse.bass2jax import _bass_exec_p, install_neuronx_cc_hook, partition_id_tensor
from concourse import library_config

F32 = mybir.dt.float32
BF16 = mybir.dt.bfloat16
I16 = mybir.dt.int16
I8 = mybir.dt.int8
U8 = mybir.dt.uint8

N, E, G = 10000, 100000, 16
NS, NV = 128, 64
DIM = NS + 3 * NV
NSP2 = 16
FC = 128
EPS = 1e-5
NCORES = 8
ET = 512
NTAB_ELEM = 384
EF_SCALE = 32.0
Z_SCALE = 24.0

AL = mybir.AluOpType
AF = mybir.ActivationFunctionType

_CACHE = {}

# m-major column permutation: mmaj(x) = x[:, COLPERM]
COLPERM = np.concatenate([
    np.arange(NS),
    *[NS + np.arange(NV) * 3 + m for m in range(3)],
]).astype(np.int64)
INV_COLPERM = np.argsort(COLPERM)

# ---------------- blob layouts (shared across cores, AllGather'ed) ----------
BLOB16 = [
    ("ntab", (N, NTAB_ELEM)),
    ("wsc_s", (128, NSP2, 128)),
    ("wsc_v01", (128, NSP2, 128)),
    ("wsc_v2", (64, NSP2, 64)),
    ("wpre0", (128, 128)),
    ("wpre1bd", (128, 128)),
    ("wpre1m2", (64, 64)),
    ("wss_a", (128, 192)),
    ("wss_b", (128, 192)),
    ("wsv_a", (128, 64)),
    ("wsv_b", (128, 64)),
    ("wvs_hi", (128, 192)),
    ("wvs_lo", (64, 192)),
    ("wvv_bdi", (128, 128)),
    ("wvv_bdj", (128, 128)),
    ("wvv_bdv", (128, 128)),
    ("wvv_ti", (128, 64)),
    ("wvv_tj", (128, 64)),
    ("wvv_tv", (64, 64)),
    ("selsh", (4, 3 * 128)),
    ("ll", (128, 64)),
    ("l2", (128, 64)),
    ("stsel", (128, 3, 3)),
]
BLOB32 = [
    ("bpre0", (128, 1)),
    ("wss_c", (128, 192)),
    ("wsv_c", (128, 64)),
    ("wf1", (128, 64)), ("bf1", (64, 1)),
    ("wf2", (64, 64)), ("bf2", (64, 1)),
    ("wf3", (64, 192)), ("bf3a", (128, 1)), ("bf3b", (64, 1)),
    ("wpost0", (128, 128)), ("bpost0", (128, 1)),
    ("wpost1bd", (128, 128)), ("wpost1m2", (64, 64)),
    ("eye3", (3, 3)),
    ("gs_c", (128, 1)), ("gv01_c", (128, 1)), ("gv2_c", (64, 1)),
    ("gsrep", (2, 128)), ("bs_col", (128, 1)),
    ("ones2", (2, 128)),
    ("eps_c", (2, 1)),
]


COL16 = 32768
COL32 = 8192


def _layout(entries, col):
    offs, off = {}, 0
    for name, shape in entries:
        offs[name] = off
        off += int(np.prod(shape))
    tot = ((off + NCORES * col - 1) // (NCORES * col)) * (NCORES * col)
    return offs, tot


OFF16, TOT16 = _layout(BLOB16, COL16)
OFF32, TOT32 = _layout(BLOB32, COL32)
SH16 = TOT16 // NCORES
SH32 = TOT32 // NCORES


def _aux_layout(nt):
    """Per-core aux byte-blob layout (one input instead of eight small ones)."""
    epc = nt * ET
    offs, off = {}, 0

    def add(name, nbytes):
        nonlocal off
        off = (off + 63) // 64 * 64
        offs[name] = off
        off += nbytes

    add("oh8", nt * NSP2 * ET)        # u8  [nt, 16*ET]
    add("shb", 4 * epc * 2)           # bf16 [4, epc]
    add("ind2", 2 * epc)              # u8  [2, epc]
    add("ind42", nt * 128 * 8)        # u8  [nt, 128, 8]
    add("gix", nt * 16 * 32 * 2)      # i16 [nt, 16, 32]
    add("gjx", nt * 16 * 32 * 2)
    add("inv_s", 2 * 4)               # f32 [2, 1]
    add("inv_v", 2 * 4)
    return offs, (off + 63) // 64 * 64


def _bd(w):
    z = np.zeros((128, 128), w.dtype)
    z[:64, :64] = w
    z[64:, 64:] = w
    return z


def _top(w):
    z = np.zeros((128, 64), w.dtype)
    z[:64, :] = w
    return z


def build_nc(nt):
    epc = nt * ET
    nc = bacc.Bacc("TRN2", target_bir_lowering=False, debug=False,
                   num_devices=NCORES)
    dt = nc.dram_tensor

    def inp(name, shape, d=F32):
        return dt(name, shape, d, kind="ExternalInput").ap()

    ef8a = inp("ef8a", [NS, epc], I8)
    ef8b = inp("ef8b", [3 * NV, epc], I8)
    el8 = inp("el8", [FC, epc], U8)
    AXO, AXB = _aux_layout(nt)
    aux8 = inp("aux8", [1, AXB], U8)
    b16_sh = inp("b16_sh", [SH16 // COL16, COL16], BF16)
    b32_sh = inp("b32_sh", [SH32 // COL32, COL32])

    def av(name, nbytes):
        return aux8[0:1, AXO[name]:AXO[name] + nbytes]

    v_oh = av("oh8", nt * NSP2 * ET).rearrange("a (t e) -> (a t) e", t=nt)
    v_shb = av("shb", 8 * epc).bitcast(BF16).rearrange("a (p f) -> (a p) f", p=4)
    v_ind2 = av("ind2", 2 * epc).rearrange("a (p f) -> (a p) f", p=2)
    v_ind42 = av("ind42", nt * 1024).rearrange("a (t r) -> (a t) r", t=nt)
    v_gix = av("gix", nt * 1024).bitcast(I16) \
        .rearrange("a (t p f) -> (a t) p f", t=nt, p=16)
    v_gjx = av("gjx", nt * 1024).bitcast(I16) \
        .rearrange("a (t p f) -> (a t) p f", t=nt, p=16)
    v_invs = av("inv_s", 8).bitcast(F32).rearrange("a (p f) -> (a p) f", p=2)
    v_invv = av("inv_v", 8).bitcast(F32).rearrange("a (p f) -> (a p) f", p=2)

    out_fm = dt("out_fm", [DIM, epc], I8, kind="ExternalOutput").ap()

    with tile.TileContext(nc) as tc:
        with (
            tc.tile_pool(name="dram", bufs=1, space="DRAM") as dp,
            tc.tile_pool(name="persist", bufs=1) as pp,
            tc.tile_pool(name="loads", bufs=2) as lp,
            tc.tile_pool(name="ohcp", bufs=1) as ocp,
            tc.tile_pool(name="gath", bufs=2) as gp,
            tc.tile_pool(name="work", bufs=1) as wp,
            tc.tile_pool(name="krn", bufs=2) as kp,
            tc.tile_pool(name="ohbp", bufs=3) as op_,
            tc.tile_pool(name="ps", bufs=4, space="PSUM") as ps,
            tc.tile_pool(name="pz", bufs=1, space="PSUM") as pz,
            tc.tile_pool(name="pst", bufs=1, space="PSUM") as pst,
        ):
            nc.gpsimd.load_library(library_config.mlp)

            # ---- AllGather the two const blobs ----
            b16i = dp.tile([SH16 // COL16, COL16], BF16)
            nc.gpsimd.dma_start(b16i[:], b16_sh)
            b16f = dp.tile([TOT16 // COL16, COL16], BF16)
            nc.gpsimd.collective_compute(
                "AllGather", AL.bypass,
                replica_groups=[list(range(NCORES))],
                ins=[b16i[:].opt()], outs=[b16f[:].opt()])
            b32i = dp.tile([SH32 // COL32, COL32], F32)
            nc.gpsimd.dma_start(b32i[:], b32_sh)
            b32f = dp.tile([TOT32 // COL32, COL32], F32)
            nc.gpsimd.collective_compute(
                "AllGather", AL.bypass,
                replica_groups=[list(range(NCORES))],
                ins=[b32i[:].opt()], outs=[b32f[:].opt()])

            flat16 = b16f[:].rearrange("a b -> (a b)").unsqueeze(0)   # [1, TOT16]
            flat32 = b32f[:].rearrange("a b -> (a b)").unsqueeze(0)   # [1, TOT32]

            def bview(name):
                for entries, offs, full in ((BLOB16, OFF16, flat16), (BLOB32, OFF32, flat32)):
                    for n_, shape in entries:
                        if n_ == name:
                            off = offs[name]
                            sz = int(np.prod(shape))
                            flat = full[0:1, off:off + sz]
                            if len(shape) == 2:
                                return flat.rearrange("a (p f) -> (a p) f", p=shape[0])
                            return flat.rearrange("a (p s u) -> (a p) s u",
                                                  p=shape[0], s=shape[1])
                raise KeyError(name)

            def load_const(name, d):
                shape = dict(BLOB16 + BLOB32)[name]
                t = pp.tile(list(shape), d, tag=name)
                nc.sync.dma_start(t[:], bview(name))
                return t

            c_wsc_s = load_const("wsc_s", BF16)
            c_wsc_v01 = load_const("wsc_v01", BF16)
            c_wsc_v2 = load_const("wsc_v2", BF16)
            c_wpre0 = load_const("wpre0", BF16)
            c_bpre0 = load_const("bpre0", F32)
            c_wpre1bd = load_const("wpre1bd", BF16)
            c_wpre1m2 = load_const("wpre1m2", BF16)
            c_wss_a = load_const("wss_a", BF16)
            c_wss_b = load_const("wss_b", BF16)
            c_wss_c = load_const("wss_c", F32)
            c_wsv_a = load_const("wsv_a", BF16)
            c_wsv_b = load_const("wsv_b", BF16)
            c_wsv_c = load_const("wsv_c", F32)
            c_wvs_hi = load_const("wvs_hi", BF16)
            c_wvs_lo = load_const("wvs_lo", BF16)
            c_wvv_bdi = load_const("wvv_bdi", BF16)
            c_wvv_bdj = load_const("wvv_bdj", BF16)
            c_wvv_bdv = load_const("wvv_bdv", BF16)
            c_wvv_ti = load_const("wvv_ti", BF16)
            c_wvv_tj = load_const("wvv_tj", BF16)
            c_wvv_tv = load_const("wvv_tv", BF16)
            c_wf1 = load_const("wf1", F32)
            c_bf1 = load_const("bf1", F32)
            c_wf2 = load_const("wf2", F32)
            c_bf2 = load_const("bf2", F32)
            c_wf3 = load_const("wf3", F32)
            c_bf3a = load_const("bf3a", F32)
            c_bf3b = load_const("bf3b", F32)
            c_wpost0 = load_const("wpost0", F32)
            c_bpost0 = load_const("bpost0", F32)
            c_wpost1bd = load_const("wpost1bd", F32)
            c_wpost1m2 = load_const("wpost1m2", F32)
            c_selsh = load_const("selsh", BF16)
            c_ll = load_const("ll", BF16)
            c_l2 = load_const("l2", BF16)
            c_stsel = load_const("stsel", BF16)
            c_eye3 = load_const("eye3", F32)
            c_gs = load_const("gs_c", F32)
            c_gv01 = load_const("gv01_c", F32)
            c_gv2 = load_const("gv2_c", F32)
            c_gsrep = load_const("gsrep", F32)
            c_bs = load_const("bs_col", F32)
            c_ones2 = load_const("ones2", F32)
            c_eps = load_const("eps_c", F32)
            c_invs = pp.tile([2, 1], F32, tag="inv_s")
            nc.sync.dma_start(c_invs[:], v_invs)
            c_invv = pp.tile([2, 1], F32, tag="inv_v")
            nc.sync.dma_start(c_invv[:], v_invv)

            ntab_view = flat16[0:1, OFF16["ntab"]:OFF16["ntab"] + N * NTAB_ELEM] \
                .rearrange("a (n k) -> (a n) k", n=N)

            z_s_all = pp.tile([128, epc], BF16, tag="z_s_all")
            z_v01_all = pp.tile([128, epc], BF16, tag="z_v01_all")
            z_v2_all = pp.tile([64, epc], BF16, tag="z_v2_all")
            stats_ps = pst.tile([2, 3], F32)

            # ================= PHASE 1 =================
            for t in range(nt):
                sl = slice(t * ET, (t + 1) * ET)

                ef8_s = lp.tile([128, ET], I8, tag="ef8_s")
                ef8_v01 = lp.tile([128, ET], I8, tag="ef8_v01")
                ef8_v2 = lp.tile([64, ET], I8, tag="ef8_v2")
                nc.sync.dma_start(ef8_s[:], ef8a[:, sl])
                nc.sync.dma_start(ef8_v01[:], ef8b[0:128, sl])
                nc.sync.dma_start(ef8_v2[:], ef8b[128:192, sl])
                efb_s = lp.tile([128, ET], BF16, tag="efb_s")
                nc.vector.tensor_scalar(efb_s[:], ef8_s[:], 1.0 / EF_SCALE, None, op0=AL.mult)
                efb_v01 = lp.tile([128, ET], BF16, tag="efb_v01")
                nc.vector.tensor_scalar(efb_v01[:], ef8_v01[:], 1.0 / EF_SCALE, None, op0=AL.mult)
                efb_v2 = lp.tile([64, ET], BF16, tag="efb_v2")
                nc.vector.tensor_scalar(efb_v2[:], ef8_v2[:], 1.0 / EF_SCALE, None, op0=AL.mult)

                el8_t = lp.tile([128, ET], U8, tag="el8_t")
                nc.sync.dma_start(el8_t[:], el8[:, sl])
                el_t = lp.tile([128, ET], F32, tag="el_t")
                nc.vector.tensor_scalar(el_t[:], el8_t[:], 1.0, None, op0=AL.mult)

                sh_t = lp.tile([4, ET], BF16, tag="sh_t")
                nc.sync.dma_start(sh_t[:], v_shb[:, sl])
                ohc8_t = ocp.tile([1, NSP2 * ET], U8, tag="ohc8_t")
                nc.sync.dma_start(ohc8_t[:], v_oh[t:t + 1, :])
                ohc_t = ocp.tile([1, NSP2 * ET], BF16, tag="ohc_t")
                nc.vector.tensor_copy(ohc_t[:], ohc8_t[:])
                i42u = lp.tile([128, 8], U8, tag="i42u")
                nc.sync.dma_start(i42u[:], v_ind42[t:t + 1, :]
                                  .rearrange("a (p f) -> (a p) f", p=128))
                ind4_t = lp.tile([128, 8], F32, tag="ind4_t")
                nc.vector.tensor_scalar(ind4_t[:], i42u[:], 1.0, None, op0=AL.mult)
                gix_t = lp.tile([128, ET // 16], I16, tag="gix_t")
                gjx_t = lp.tile([128, ET // 16], I16, tag="gjx_t")
                for rr in range(8):
                    nc.sync.dma_start(gix_t[16 * rr:16 * (rr + 1), :], v_gix[t, :, :])
                    nc.sync.dma_start(gjx_t[16 * rr:16 * (rr + 1), :], v_gjx[t, :, :])

                # gathers (feature-major bf16 [128, 3, ET])
                gi = gp.tile([128, 3, ET], BF16, tag="gi")
                nc.gpsimd.dma_gather(gi[:], ntab_view, gix_t[:], ET, ET, NTAB_ELEM,
                                     transpose=True, single_packet=False)
                gj = gp.tile([128, 3, ET], BF16, tag="gj")
                nc.gpsimd.dma_gather(gj[:], ntab_view, gjx_t[:], ET, ET, NTAB_ELEM,
                                     transpose=True, single_packet=False)

                # sh broadcast tiles (PE sel-matmul -> psum -> bf16 sbuf)
                shb01 = wp.tile([128, ET], BF16, tag="shb01")
                shb2 = wp.tile([128, ET], BF16, tag="shb2")
                sh0b = wp.tile([128, ET], BF16, tag="sh0b")
                for k, dst in enumerate((shb01, shb2, sh0b)):
                    p = ps.tile([128, ET], F32, tag="pt")
                    nc.tensor.matmul(p[:], c_selsh[:, k * 128:(k + 1) * 128], sh_t[:],
                                     start=True, stop=True)
                    nc.scalar.copy(dst[:], p[:])

                # lin_pre
                p = ps.tile([128, ET], F32, tag="pt")
                nc.tensor.matmul(p[:], c_wpre0[:], efb_s[:], start=True, stop=True)
                s_sb = wp.tile([128, ET], F32, tag="s_sb")
                nc.scalar.activation(s_sb[:], p[:], AF.Identity, bias=c_bpre0[:, 0:1])
                p = ps.tile([128, ET], F32, tag="pt")
                nc.tensor.matmul(p[:], c_wpre1bd[:], efb_v01[:], start=True, stop=True)
                v01_sb = wp.tile([128, ET], BF16, tag="v01_sb")
                nc.scalar.copy(v01_sb[:], p[:])
                p2 = ps.tile([64, ET], F32, tag="pt")
                nc.tensor.matmul(p2[:], c_wpre1m2[:], efb_v2[:], start=True, stop=True)
                v2_sb = wp.tile([64, ET], BF16, tag="v2_sb")
                nc.scalar.copy(v2_sb[:], p2[:])

                # radial MLP (1/255 folded into wf1)
                p2 = ps.tile([64, ET], F32, tag="pt")
                nc.tensor.matmul(p2[:], c_wf1[:], el_t[:], start=True, stop=True)
                h1 = wp.tile([64, ET], F32, tag="h1")
                nc.scalar.activation(h1[:], p2[:], AF.Silu, bias=c_bf1[:, 0:1])
                p2 = ps.tile([64, ET], F32, tag="pt")
                nc.tensor.matmul(p2[:], c_wf2[:], h1[:], start=True, stop=True)
                h2 = wp.tile([64, ET], F32, tag="gate")
                nc.scalar.activation(h2[:], p2[:], AF.Silu, bias=c_bf2[:, 0:1])
                p = ps.tile([128, ET], F32, tag="pt")
                nc.tensor.matmul(p[:], c_wf3[:, 0:128], h2[:], start=True, stop=True)
                w_s = wp.tile([128, ET], F32, tag="w_s")
                nc.scalar.activation(w_s[:], p[:], AF.Identity, bias=c_bf3a[:, 0:1])
                p2 = ps.tile([64, ET], F32, tag="pt")
                nc.tensor.matmul(p2[:], c_wf3[:, 128:192], h2[:], start=True, stop=True)
                w_v = wp.tile([64, ET], F32, tag="w_v")
                nc.scalar.activation(w_v[:], p2[:], AF.Identity, bias=c_bf3b[:, 0:1])

                # FCTP self-connection -> accumulate into z psums
                z_s_ps = pz.tile([128, ET], F32, tag="z_s_ps")
                z_v01_ps = pz.tile([128, ET], F32, tag="z_v01_ps")
                z_v2_ps = pz.tile([64, ET], F32, tag="z_v2_ps")
                ohv = ohc_t[:].rearrange("a (s e) -> a s e", s=NSP2)
                for s in range(NSP2):
                    ohb = op_.tile([128, ET], BF16, tag="ohb")
                    nc.gpsimd.partition_broadcast(ohb[:], ohv[0:1, s, :])
                    kr = kp.tile([128, ET], BF16, tag="kr_s")
                    nc.vector.tensor_tensor(kr[:], efb_s[:], ohb[:], op=AL.mult)
                    nc.tensor.matmul(z_s_ps[:], c_wsc_s[:, s, :], kr[:],
                                     start=(s == 0), stop=False)
                    krv = kp.tile([128, ET], BF16, tag="kr_v")
                    nc.vector.tensor_tensor(krv[:], efb_v01[:], ohb[:], op=AL.mult)
                    nc.tensor.matmul(z_v01_ps[:], c_wsc_v01[:, s, :], krv[:],
                                     start=(s == 0), stop=False)
                    kr2 = kp.tile([64, ET], BF16, tag="kr_2")
                    nc.vector.tensor_tensor(kr2[:], efb_v2[:], ohb[0:64, :], op=AL.mult)
                    nc.tensor.matmul(z_v2_ps[:], c_wsc_v2[:, s, :], kr2[:],
                                     start=(s == 0), stop=False)

                # d = sum_m v_in_m * sh1_m   (192 rows: [d_i; d_j; d_v])
                prods = []
                for (src, tag) in ((gi[:, 1, :], "pd1"), (gj[:, 1, :], "pd3"),
                                   (v01_sb[:], "pd5")):
                    pr = wp.tile([128, ET], BF16, tag=tag)
                    nc.vector.tensor_tensor(pr[:], src, shb01[:], op=AL.mult)
                    prods.append(pr)
                prods2 = []
                for (src, tag) in ((gi[:, 2, :], "pd2"), (gj[:, 2, :], "pd4")):
                    pr = wp.tile([128, ET], BF16, tag=tag)
                    nc.vector.tensor_tensor(pr[:], src, shb2[:], op=AL.mult)
                    prods2.append(pr)
                pr6 = wp.tile([64, ET], BF16, tag="pd6")
                nc.vector.tensor_tensor(pr6[:], v2_sb[:], shb2[0:64, :], op=AL.mult)

                d_i = ps.tile([64, ET], F32, tag="pt")
                nc.tensor.matmul(d_i[:], c_ll[:], prods[0][:], start=True, stop=False)
                nc.tensor.matmul(d_i[:], c_l2[:], prods2[0][:], start=False, stop=True)
                d_j = ps.tile([64, ET], F32, tag="pt")
                nc.tensor.matmul(d_j[:], c_ll[:], prods[1][:], start=True, stop=False)
                nc.tensor.matmul(d_j[:], c_l2[:], prods2[1][:], start=False, stop=True)
                d_v = ps.tile([64, ET], F32, tag="pt")
                nc.tensor.matmul(d_v[:], c_ll[:], prods[2][:], start=True, stop=False)
                nc.tensor.matmul(d_v[:], c_l2[0:64, :], pr6[:], start=False, stop=True)
                d1 = wp.tile([128, ET], BF16, tag="d1")
                nc.scalar.copy(d1[0:64, :], d_i[:])
                nc.scalar.copy(d1[64:128, :], d_j[:])
                d2 = wp.tile([64, ET], BF16, tag="d2")
                nc.scalar.copy(d2[:], d_v[:])

                # out_s = sh0*(s_in @ Wss) + d @ Wvs
                os1a = ps.tile([128, ET], F32, tag="pt")
                nc.tensor.matmul(os1a[:], c_wss_a[:, 0:128], gi[:, 0, :], start=True, stop=False)
                nc.tensor.matmul(os1a[:], c_wss_b[:, 0:128], gj[:, 0, :], start=False, stop=False)
                nc.tensor.matmul(os1a[:], c_wss_c[:, 0:128], s_sb[:], start=False, stop=True)
                os1b = ps.tile([64, ET], F32, tag="pt")
                nc.tensor.matmul(os1b[:], c_wss_a[:, 128:192], gi[:, 0, :], start=True, stop=False)
                nc.tensor.matmul(os1b[:], c_wss_b[:, 128:192], gj[:, 0, :], start=False, stop=False)
                nc.tensor.matmul(os1b[:], c_wss_c[:, 128:192], s_sb[:], start=False, stop=True)
                os2a = ps.tile([128, ET], F32, tag="pt")
                nc.tensor.matmul(os2a[:], c_wvs_hi[:, 0:128], d1[:], start=True, stop=False)
                nc.tensor.matmul(os2a[:], c_wvs_lo[:, 0:128], d2[:], start=False, stop=True)
                os2b = ps.tile([64, ET], F32, tag="pt")
                nc.tensor.matmul(os2b[:], c_wvs_hi[:, 128:192], d1[:], start=True, stop=False)
                nc.tensor.matmul(os2b[:], c_wvs_lo[:, 128:192], d2[:], start=False, stop=True)

                osA = wp.tile([128, ET], F32, tag="osA")
                nc.vector.tensor_tensor(osA[:], os1a[:], sh0b[:], op=AL.mult)
                nc.vector.tensor_tensor(osA[:], osA[:], os2a[:], op=AL.add)
                osB = wp.tile([64, ET], F32, tag="osB")
                nc.vector.tensor_tensor(osB[:], os1b[:], sh0b[0:64, :], op=AL.mult)
                nc.vector.tensor_tensor(osB[:], osB[:], os2b[:], op=AL.add)

                zs_g = wp.tile([128, ET], F32, tag="zs_g")
                nc.scalar.activation(zs_g[:], osA[:], AF.Silu)
                gate = wp.tile([64, ET], F32, tag="gate")
                nc.scalar.activation(gate[:], osB[:], AF.Sigmoid)

                # out_v = sh1_m*(s_in @ Wsv) + sh0*(v_in_m @ Wvv)
                q_ps = ps.tile([64, ET], F32, tag="pt")
                nc.tensor.matmul(q_ps[:], c_wsv_a[:], gi[:, 0, :], start=True, stop=False)
                nc.tensor.matmul(q_ps[:], c_wsv_b[:], gj[:, 0, :], start=False, stop=False)
                nc.tensor.matmul(q_ps[:], c_wsv_c[:], s_sb[:], start=False, stop=True)
                t2v01 = ps.tile([128, ET], F32, tag="pt")
                nc.tensor.matmul(t2v01[:], c_wvv_bdi[:], gi[:, 1, :], start=True, stop=False)
                nc.tensor.matmul(t2v01[:], c_wvv_bdj[:], gj[:, 1, :], start=False, stop=False)
                nc.tensor.matmul(t2v01[:], c_wvv_bdv[:], v01_sb[:], start=False, stop=True)
                t2v2 = ps.tile([64, ET], F32, tag="pt")
                nc.tensor.matmul(t2v2[:], c_wvv_ti[:], gi[:, 2, :], start=True, stop=False)
                nc.tensor.matmul(t2v2[:], c_wvv_tj[:], gj[:, 2, :], start=False, stop=False)
                nc.tensor.matmul(t2v2[:], c_wvv_tv[:], v2_sb[:], start=False, stop=True)

                qd = wp.tile([128, ET], F32, tag="qd")
                nc.scalar.copy(qd[0:64, :], q_ps[:])
                nc.scalar.copy(qd[64:128, :], q_ps[:])
                gw = wp.tile([64, ET], F32, tag="gw")
                nc.vector.tensor_tensor(gw[:], gate[:], w_v[:], op=AL.mult)
                gwd = wp.tile([128, ET], F32, tag="gwd")
                nc.scalar.copy(gwd[0:64, :], gw[:])
                nc.scalar.copy(gwd[64:128, :], gw[:])

                ov01 = wp.tile([128, ET], F32, tag="ov01")
                nc.vector.tensor_tensor(ov01[:], qd[:], shb01[:], op=AL.mult)
                tmp01 = wp.tile([128, ET], F32, tag="tmp01")
                nc.vector.tensor_tensor(tmp01[:], t2v01[:], sh0b[:], op=AL.mult)
                nc.vector.tensor_tensor(ov01[:], ov01[:], tmp01[:], op=AL.add)
                nc.vector.tensor_tensor(ov01[:], ov01[:], gwd[:], op=AL.mult)
                ov2 = wp.tile([64, ET], F32, tag="ov2")
                nc.vector.tensor_tensor(ov2[:], q_ps[:], shb2[0:64, :], op=AL.mult)
                tmp2 = wp.tile([64, ET], F32, tag="tmp2")
                nc.vector.tensor_tensor(tmp2[:], t2v2[:], sh0b[0:64, :], op=AL.mult)
                nc.vector.tensor_tensor(ov2[:], ov2[:], tmp2[:], op=AL.add)
                nc.vector.tensor_tensor(ov2[:], ov2[:], gw[:], op=AL.mult)

                zs_w = wp.tile([128, ET], F32, tag="zs_w")
                nc.vector.tensor_tensor(zs_w[:], zs_g[:], w_s[:], op=AL.mult)

                # lin_post accumulates onto the FCTP psums
                nc.tensor.matmul(z_s_ps[:], c_wpost0[:], zs_w[:], start=False, stop=True)
                nc.tensor.matmul(z_v01_ps[:], c_wpost1bd[:], ov01[:], start=False, stop=True)
                nc.tensor.matmul(z_v2_ps[:], c_wpost1m2[:], ov2[:], start=False, stop=True)

                nc.scalar.activation(z_s_all[:, sl], z_s_ps[:], AF.Identity,
                                     bias=c_bpost0[:, 0:1])
                nc.scalar.copy(z_v01_all[:, sl], z_v01_ps[:])
                nc.scalar.copy(z_v2_all[:, sl], z_v2_ps[:])

                # stats: [sum(z_s); sum(z_s^2); sum(z_v^2)] per edge -> per graph
                sqs = wp.tile([128, ET], BF16, tag="zs_w")
                nc.scalar.activation(sqs[:], z_s_all[:, sl], AF.Square)
                sqv01 = wp.tile([128, ET], BF16, tag="tmp01")
                nc.scalar.activation(sqv01[:], z_v01_all[:, sl], AF.Square)
                sqv2 = wp.tile([64, ET], BF16, tag="tmp2")
                nc.scalar.activation(sqv2[:], z_v2_all[:, sl], AF.Square)
                st_ps = ps.tile([3, ET], F32, tag="pt")
                nc.tensor.matmul(st_ps[:], c_stsel[:, 0, :], z_s_all[:, sl], start=True, stop=False)
                nc.tensor.matmul(st_ps[:], c_stsel[:, 1, :], sqs[:], start=False, stop=False)
                nc.tensor.matmul(st_ps[:], c_stsel[:, 2, :], sqv01[:], start=False, stop=False)
                nc.tensor.matmul(st_ps[:], c_stsel[0:64, 2, :], sqv2[:], start=False, stop=True)
                st_sb = wp.tile([3, ET], F32, tag="gw")
                nc.vector.tensor_copy(st_sb[:], st_ps[:])
                for c in range(4):
                    tp_ps = ps.tile([128, 3], F32, tag="pt")
                    nc.tensor.transpose(tp_ps[:], st_sb[:, c * 128:(c + 1) * 128],
                                        c_eye3[:])
                    tp_sb = wp.tile([128, 3], F32, tag="tp_sb")
                    nc.vector.tensor_copy(tp_sb[:], tp_ps[:])
                    nc.tensor.matmul(stats_ps[:], ind4_t[:, 2 * c:2 * (c + 1)], tp_sb[:],
                                     start=(t == 0 and c == 0), stop=(t == nt - 1 and c == 3))

            # ============ stats finalize ============
            st = pp.tile([2, 3], F32, tag="st_fin")
            nc.vector.tensor_copy(st[:], stats_ps[:])
            mean = pp.tile([2, 1], F32, tag="mean")
            nc.vector.tensor_scalar(mean[:], st[:, 0:1], c_invs[:, 0:1], None, op0=AL.mult)
            es2 = pp.tile([2, 1], F32, tag="es2")
            nc.vector.tensor_scalar(es2[:], st[:, 1:2], c_invs[:, 0:1], None, op0=AL.mult)
            var_s = pp.tile([2, 1], F32, tag="var_s")
            nc.vector.tensor_tensor(var_s[:], mean[:], mean[:], op=AL.mult)
            nc.vector.tensor_tensor(var_s[:], es2[:], var_s[:], op=AL.subtract)
            var_v = pp.tile([2, 1], F32, tag="var_v")
            nc.vector.tensor_scalar(var_v[:], st[:, 2:3], c_invv[:, 0:1], None, op0=AL.mult)
            rstd_s = pp.tile([2, 1], F32, tag="rstd_s")
            nc.scalar.activation(rstd_s[:], var_s[:], AF.Sqrt, bias=c_eps[:, 0:1])
            nc.vector.reciprocal(rstd_s[:], rstd_s[:])
            rstd_v = pp.tile([2, 1], F32, tag="rstd_v")
            nc.scalar.activation(rstd_v[:], var_v[:], AF.Sqrt, bias=c_eps[:, 0:1])
            nc.vector.reciprocal(rstd_v[:], rstd_v[:])

            a_l = pp.tile([2, 128], F32, tag="a_l")
            nc.vector.tensor_scalar(a_l[:], c_ones2[:], rstd_s[:, 0:1], None, op0=AL.mult)
            mrn = pp.tile([2, 1], F32, tag="mrn")
            nc.vector.tensor_scalar(mrn[:], mean[:], rstd_s[:, 0:1], -1.0,
                                    op0=AL.mult, op1=AL.mult)
            b_l = pp.tile([2, 128], F32, tag="b_l")
            nc.vector.tensor_scalar(b_l[:, :], c_gsrep[:], mrn[:, 0:1], None, op0=AL.mult)
            cc_l = pp.tile([2, 128], F32, tag="cc_l")
            nc.vector.tensor_scalar(cc_l[:], c_ones2[:], rstd_v[:, 0:1], None, op0=AL.mult)

            # ================= PHASE 2 =================
            for t in range(nt):
                sl = slice(t * ET, (t + 1) * ET)
                ind_t8 = lp.tile([2, ET], U8, tag="ind_t8")
                nc.sync.dma_start(ind_t8[:], v_ind2[:, sl])
                ind_t = lp.tile([2, ET], F32, tag="ind_t")
                nc.vector.tensor_scalar(ind_t[:], ind_t8[:], 1.0, None, op0=AL.mult)
                a_ps = ps.tile([128, ET], F32, tag="pt")
                nc.tensor.matmul(a_ps[:], a_l[:], ind_t[:], start=True, stop=True)
                b_ps = ps.tile([128, ET], F32, tag="pt")
                nc.tensor.matmul(b_ps[:], b_l[:], ind_t[:], start=True, stop=True)
                c_ps = ps.tile([128, ET], F32, tag="pt")
                nc.tensor.matmul(c_ps[:], cc_l[:], ind_t[:], start=True, stop=True)

                # a/b/c rows carry the x24 int8 output scale (folded on host)
                res_s = wp.tile([128, ET], F32, tag="osA")
                nc.vector.scalar_tensor_tensor(res_s[:], z_s_all[:, sl], c_gs[:, 0:1],
                                               a_ps[:], op0=AL.mult, op1=AL.mult)
                res_sb = wp.tile([128, ET], I8, tag="qd")
                nc.vector.scalar_tensor_tensor(res_sb[:], b_ps[:], c_bs[:, 0:1],
                                               res_s[:], op0=AL.add, op1=AL.add)
                res_v01 = wp.tile([128, ET], I8, tag="ov01")
                nc.vector.scalar_tensor_tensor(res_v01[:], z_v01_all[:, sl], c_gv01[:, 0:1],
                                               c_ps[:], op0=AL.mult, op1=AL.mult)
                res_v2 = wp.tile([64, ET], I8, tag="ov2")
                nc.vector.scalar_tensor_tensor(res_v2[:], z_v2_all[:, sl], c_gv2[:, 0:1],
                                               c_ps[0:64, :], op0=AL.mult, op1=AL.mult)

                nc.sync.dma_start(out_fm[0:128, sl], res_sb[:])
                nc.sync.dma_start(out_fm[128:256, sl], res_v01[:])
                nc.sync.dma_start(out_fm[256:320, sl], res_v2[:])

    nc.compile()
    return nc


class SpmdRunner:
    """Persistent-jit shard_map runner (outputs materialized on device)."""

    def __init__(self, nc, n_cores=NCORES):
        install_neuronx_cc_hook()
        self.nc = nc
        self.n_cores = n_cores
        self.partition_name = nc.partition_id_tensor.name if nc.partition_id_tensor else None
        in_names, out_names, out_avals = [], [], []
        for alloc in nc.m.functions[0].allocations:
            if not isinstance(alloc, mybir.MemoryLocationSet):
                continue
            name = alloc.memorylocations[0].name
            if alloc.kind == "ExternalInput":
                if name != self.partition_name:
                    in_names.append(name)
            elif alloc.kind == "ExternalOutput":
                out_names.append(name)
                out_avals.append(jax.core.ShapedArray(
                    tuple(alloc.tensor_shape), mybir.dt.np(alloc.dtype)))
        self.in_names, self.out_names, self.out_avals = in_names, out_names, out_avals
        n_params, n_outs = len(in_names), len(out_avals)
        all_in = in_names + out_names + ([self.partition_name] if self.partition_name else [])

        devices = jax.devices()[:n_cores]
        self.mesh = Mesh(np.asarray(devices), ("core",))
        self.sh_core = NamedSharding(self.mesh, PartitionSpec("core"))
        nc_ = nc
        avals = tuple(out_avals)
        pname = self.partition_name

        def _body(*args):
            operands = list(args)
            if pname is not None:
                operands.append(partition_id_tensor())
            return tuple(_bass_exec_p.bind(
                *operands,
                out_avals=avals,
                in_names=tuple(all_in),
                out_names=tuple(out_names),
                lowering_input_output_aliases=(),
                sim_require_finite=True,
                sim_require_nnan=True,
                nc=nc_,
            ))

        in_specs = (PartitionSpec("core"),) * (n_params + n_outs)
        out_specs = (PartitionSpec("core"),) * n_outs
        self.sharded = jax.jit(
            shard_map(_body, mesh=self.mesh, in_specs=in_specs,
                      out_specs=out_specs, check_rep=False),
            donate_argnums=tuple(range(n_params, n_params + n_outs)),
            keep_unused=True,
        )
        zshapes = [(n_cores * a.shape[0], *a.shape[1:]) for a in out_avals]
        zdtypes = [a.dtype for a in out_avals]
        self._mk_zeros = jax.jit(
            lambda: tuple(jnp.zeros(s, d) for s, d in zip(zshapes, zdtypes)),
            out_shardings=tuple(self.sh_core for _ in out_avals),
        )

    def __call__(self, concat_map):
        zeros = self._mk_zeros()
        outs = self.sharded(*[concat_map[n] for n in self.in_names], *zeros)
        res = [np.asarray(o) for o in outs]
        return {
            n: res[i].reshape(self.n_cores, *self.out_avals[i].shape)
            for i, n in enumerate(self.out_names)
        }


def compute_sharding(inputs):
    """Graph->core assignment and the edge permutation."""
    edge_index = np.asarray(inputs["edge_index"]).astype(np.int64)
    batch = np.asarray(inputs["batch"]).astype(np.int64)
    i_idx, j_idx = edge_index[0], edge_index[1]
    batch_edge = batch[i_idx]

    cnt_edges = np.bincount(batch_edge, minlength=G)
    order = np.argsort(-cnt_edges)
    pairs = [(order[k], order[G - 1 - k]) for k in range(G // 2)]
    core_of_graph = np.zeros(G, np.int64)
    local_of_graph = np.zeros(G, np.int64)
    for c, (g1, g2) in enumerate(pairs):
        core_of_graph[g1] = c
        core_of_graph[g2] = c
        local_of_graph[g1] = 0
        local_of_graph[g2] = 1
    core_of_edge = core_of_graph[batch_edge]
    perm = np.argsort(core_of_edge, kind="stable")
    counts = np.bincount(core_of_edge, minlength=NCORES)
    starts = np.zeros(NCORES + 1, np.int64)
    starts[1:] = np.cumsum(counts)
    return dict(i_idx=i_idx, j_idx=j_idx, batch_edge=batch_edge,
                cnt_edges=cnt_edges, pairs=pairs, local_of_graph=local_of_graph,
                perm=perm, counts=counts, starts=starts)


def build_blobs(inputs):
    """The shared (AllGather'ed) weight/node-table blobs."""
    node_fea = np.asarray(inputs["node_fea"], np.float32)
    sq2 = math.sqrt(2.0)
    Wv = {}
    ntab = np.zeros((N, NTAB_ELEM), np.float32)
    ntab[:, :DIM] = node_fea[:, COLPERM]
    Wv["ntab"] = ntab
    Wv["wsc_s"] = np.asarray(inputs["Wsc_s"], np.float32) / (math.sqrt(NS * NSP2) * 255.0)
    wv = np.asarray(inputs["Wsc_v"], np.float32) / (math.sqrt(NV * NSP2) * 255.0)
    Wv["wsc_v01"] = np.stack([_bd(wv[:, s, :]) for s in range(NSP2)], axis=1)
    Wv["wsc_v2"] = wv
    Wv["wpre0"] = np.asarray(inputs["Wpre0"], np.float32) / math.sqrt(NS)
    Wv["bpre0"] = np.asarray(inputs["bpre0"], np.float32).reshape(128, 1)
    wpre1 = np.asarray(inputs["Wpre1"], np.float32) / math.sqrt(NV)
    Wv["wpre1bd"] = _bd(wpre1)
    Wv["wpre1m2"] = wpre1
    wss = np.asarray(inputs["Wss"], np.float32) / (math.sqrt(3 * NS) * sq2)
    Wv["wss_a"] = wss[0:128]
    Wv["wss_b"] = wss[128:256]
    Wv["wss_c"] = wss[256:384]
    wsv = np.asarray(inputs["Wsv"], np.float32) / (math.sqrt(3 * NS) * sq2)
    Wv["wsv_a"] = wsv[0:128]
    Wv["wsv_b"] = wsv[128:256]
    Wv["wsv_c"] = wsv[256:384]
    wvs_full = np.asarray(inputs["Wvs"], np.float32) / (math.sqrt(9 * NV) * sq2)
    Wv["wvs_hi"] = wvs_full[0:128]
    Wv["wvs_lo"] = wvs_full[128:192]
    wvv = np.asarray(inputs["Wvv"], np.float32) / (math.sqrt(3 * NV) * sq2)
    Wv["wvv_bdi"] = _bd(wvv[0:64])
    Wv["wvv_bdj"] = _bd(wvv[64:128])
    Wv["wvv_bdv"] = _bd(wvv[128:192])
    Wv["wvv_ti"] = _top(wvv[0:64])
    Wv["wvv_tj"] = _top(wvv[64:128])
    Wv["wvv_tv"] = wvv[128:192]
    Wv["wf1"] = np.asarray(inputs["Wf1"], np.float32) / 255.0
    Wv["bf1"] = np.asarray(inputs["bf1"], np.float32).reshape(64, 1)
    Wv["wf2"] = np.asarray(inputs["Wf2"], np.float32)
    Wv["bf2"] = np.asarray(inputs["bf2"], np.float32).reshape(64, 1)
    Wv["wf3"] = np.asarray(inputs["Wf3"], np.float32)
    bf3 = np.asarray(inputs["bf3"], np.float32)
    Wv["bf3a"] = bf3[0:128].reshape(128, 1)
    Wv["bf3b"] = bf3[128:192].reshape(64, 1)
    Wv["wpost0"] = np.asarray(inputs["Wpost0"], np.float32) / math.sqrt(NS)
    Wv["bpost0"] = np.asarray(inputs["bpost0"], np.float32).reshape(128, 1)
    wpost1 = np.asarray(inputs["Wpost1"], np.float32) / math.sqrt(NV)
    Wv["wpost1bd"] = _bd(wpost1)
    Wv["wpost1m2"] = wpost1
    selsh = np.zeros((4, 3 * 128), np.float32)
    selsh[1, 0:64] = 1.0
    selsh[2, 64:128] = 1.0
    selsh[3, 128:192] = 1.0
    selsh[0, 256:384] = 1.0
    Wv["selsh"] = selsh
    i64 = np.eye(64, dtype=np.float32)
    Wv["ll"] = np.vstack([i64, i64])
    Wv["l2"] = np.vstack([i64, np.zeros((64, 64), np.float32)])
    stsel = np.zeros((128, 3, 3), np.float32)
    stsel[:, 0, 0] = 1.0
    stsel[:, 1, 1] = 1.0
    stsel[:, 2, 2] = 1.0
    Wv["stsel"] = stsel
    Wv["eye3"] = np.eye(3, dtype=np.float32)
    gamma_s = np.asarray(inputs["gamma_s"], np.float32)
    beta_s = np.asarray(inputs["beta_s"], np.float32)
    gamma_v = np.asarray(inputs["gamma_v"], np.float32)
    # ones2/gsrep/bs_col carry the x Z_SCALE fold for the int8 z output
    Wv["gs_c"] = gamma_s.reshape(128, 1)
    Wv["gv01_c"] = np.concatenate([gamma_v, gamma_v]).reshape(128, 1)
    Wv["gv2_c"] = gamma_v.reshape(64, 1)
    Wv["gsrep"] = Z_SCALE * np.tile(gamma_s[None, :], (2, 1))
    Wv["bs_col"] = Z_SCALE * beta_s.reshape(128, 1)
    Wv["ones2"] = np.full((2, 128), Z_SCALE, np.float32)
    Wv["eps_c"] = np.full((2, 1), EPS, np.float32)

    b16 = np.zeros((TOT16,), ml_dtypes.bfloat16)
    for name, shape in BLOB16:
        off = OFF16[name]
        b16[off:off + int(np.prod(shape))] = \
            Wv[name].astype(ml_dtypes.bfloat16).reshape(-1)
    b32 = np.zeros((TOT32,), np.float32)
    for name, shape in BLOB32:
        off = OFF32[name]
        b32[off:off + int(np.prod(shape))] = Wv[name].astype(np.float32).reshape(-1)
    return (b16.reshape(NCORES * (SH16 // COL16), COL16),
            b32.reshape(NCORES * (SH32 // COL32), COL32))


def get_runner(nt):
    key = ("runner", nt)
    if key not in _CACHE:
        nc = build_nc(nt)
        _CACHE[key] = SpmdRunner(nc)
    return _CACHE[key]


class _Res:
    exec_time_ns = None


def _scratch(name, shape, dtype):
    key = ("scratch", name, shape, str(dtype))
    if key not in _CACHE:
        _CACHE[key] = np.zeros(shape, dtype)
    return _CACHE[key]


def run(inputs, trace=False):
    sh = compute_sharding(inputs)
    maxpair = int(sh["counts"].max())
    nt = max(25, -(-maxpair // ET))
    epc = nt * ET
    assert sh["counts"].max() <= epc
    runner = get_runner(nt)
    shc = runner.sh_core
    dev = {}

    def put(name, arr):
        dev[name] = jax.device_put(arr, shc)

    zeros = runner._mk_zeros()          # async, on device

    # ---- ship as soon as each piece is packed (transfer overlaps packing) --
    b16_sh, b32_sh = build_blobs(inputs)
    put("b16_sh", b16_sh)
    put("b32_sh", b32_sh)

    perm, starts = sh["perm"], sh["starts"]
    core_perms = [perm[starts[c]:starts[c + 1]] for c in range(NCORES)]

    el = np.asarray(inputs["edge_length_embedded"], np.float32)
    elf = _scratch("elf", (E, FC), np.float32)
    np.multiply(el, 255.0, out=elf)
    np.rint(elf, out=elf)
    np.clip(elf, 0, 255, out=elf)
    elq = elf.astype(np.uint8)
    el8c = _scratch("el8", (NCORES * FC, epc), np.uint8)
    for c in range(NCORES):
        pidx = core_perms[c]
        ne = len(pidx)
        el8c[c * FC:(c + 1) * FC, :ne] = elq[pidx].T
        el8c[c * FC:(c + 1) * FC, ne:] = 0
    put("el8", el8c)

    ohq = np.rint(np.asarray(inputs["edge_one_hot"], np.float32) * 255.0).astype(np.uint8)
    shb16 = np.asarray(inputs["edge_sh"], np.float32).astype(ml_dtypes.bfloat16)
    AXO, AXB = _aux_layout(nt)
    auxc = _scratch("aux8", (NCORES * 1, AXB), np.uint8)
    auxc[:] = 0

    def ax(c, name, nbytes):
        return auxc[c, AXO[name]:AXO[name] + nbytes]

    for c in range(NCORES):
        pidx = core_perms[c]
        ne = len(pidx)
        ohc = np.zeros((epc, NSP2), np.uint8)
        ohc[:ne] = ohq[pidx]
        ax(c, "oh8", nt * NSP2 * ET)[:] = \
            ohc.reshape(nt, ET, NSP2).transpose(0, 2, 1).reshape(-1)
        shbc = np.zeros((4, epc), ml_dtypes.bfloat16)
        shbc[:, :ne] = shb16[pidx].T
        ax(c, "shb", 8 * epc)[:] = shbc.view(np.uint8).reshape(-1)
        lg = sh["local_of_graph"][sh["batch_edge"][pidx]]
        oh2 = np.zeros((epc, 2), np.uint8)
        oh2[np.arange(ne), lg] = 1
        ax(c, "ind2", 2 * epc)[:] = oh2.T.reshape(-1)
        ax(c, "ind42", nt * 1024)[:] = \
            oh2.reshape(nt, 4, 128, 2).transpose(0, 2, 1, 3).reshape(-1)
        iic = np.zeros((epc,), np.int16)
        jjc = np.zeros((epc,), np.int16)
        iic[:ne] = sh["i_idx"][pidx]
        jjc[:ne] = sh["j_idx"][pidx]
        ax(c, "gix", nt * 1024)[:] = \
            iic.reshape(nt, ET // 16, 16).transpose(0, 2, 1).copy().view(np.uint8).reshape(-1)
        ax(c, "gjx", nt * 1024)[:] = \
            jjc.reshape(nt, ET // 16, 16).transpose(0, 2, 1).copy().view(np.uint8).reshape(-1)
        g1, g2 = sh["pairs"][c]
        cl = np.maximum(np.array([sh["cnt_edges"][g1], sh["cnt_edges"][g2]], np.float32), 1.0)
        ax(c, "inv_s", 8)[:] = (1.0 / (cl * NS)).astype(np.float32).view(np.uint8)
        ax(c, "inv_v", 8)[:] = (1.0 / (cl * NV * 3)).astype(np.float32).view(np.uint8)
    put("aux8", auxc)
    edge_fea = np.asarray(inputs["edge_fea"], np.float32)
    efqf = _scratch("efqf", (E, DIM), np.float32)
    np.multiply(edge_fea, EF_SCALE, out=efqf)
    np.rint(efqf, out=efqf)
    np.clip(efqf, -127, 127, out=efqf)
    efq = efqf.astype(np.int8)
    ef8ac = _scratch("ef8a", (NCORES * NS, epc), np.int8)
    for c in range(NCORES):
        pidx = core_perms[c]
        ne = len(pidx)
        ef8ac[c * NS:(c + 1) * NS, :ne] = efq[pidx, :NS].T
        ef8ac[c * NS:(c + 1) * NS, ne:] = 0
    put("ef8a", ef8ac)
    ef8bc = _scratch("ef8b", (NCORES * 3 * NV, epc), np.int8)
    vperm = COLPERM[NS:]
    for c in range(NCORES):
        pidx = core_perms[c]
        ne = len(pidx)
        ef8bc[c * 3 * NV:(c + 1) * 3 * NV, :ne] = efq[np.ix_(pidx, vperm)].T
        ef8bc[c * 3 * NV:(c + 1) * 3 * NV, ne:] = 0
    put("ef8b", ef8bc)


    outs = runner.sharded(*[dev[n] for n in runner.in_names], *zeros)

    # ---- fetch per-core shards; skip-add overlaps the remaining downloads --
    out_g = outs[0]
    shard_by_core = {}
    for s in out_g.addressable_shards:
        shard_by_core[s.index[0].start // DIM] = s.data
    out = np.empty((E, DIM), np.float32)
    from concurrent.futures import ThreadPoolExecutor
    with ThreadPoolExecutor(2) as ex:
        futs = [ex.submit(np.asarray, shard_by_core[c]) for c in range(NCORES)]
        for c in range(NCORES):
            pidx = core_perms[c]
            blk = futs[c].result()[:, :len(pidx)]   # [320, ne] m-major int8 (x24)
            rows = blk[INV_COLPERM, :].T.astype(np.float32)
            rows *= (1.0 / Z_SCALE)
            out[pidx] = edge_fea[pidx] + rows
    return out, _Res()


def kernel(**inputs) -> np.ndarray:
    out, _ = run(inputs, trace=False)
    return out
